# revision 12
# baseline (speedup 1.0000x reference)
"""Trainium2 Bass kernel: masked-bilinear channel-mixing Conv2d (eigen form).

reference math (N=4, C=96, H=W=32, O=96, K=3, PAD=1):
    p = avgpool3x3(x, count_include_pad) -> [N, C, H, W] -> [N, L, C]
    wm = weight * mask                              [O, C, C]
    y[n,l,o] = p_l^T wm_o p_l + bias[o]

Host folds the weights: S_o = wm_o + wm_o^T (zero diagonal), eigh ->
(lam, V); U_or = sqrt(|lam|/2)/9 * v_or and s_or = sign(lam_or).  Then
    y[o,l] = sum_r s_or * (U_or . boxsum_l)^2 + bias[o]
which needs NO elementwise tensor-tensor product on device - only a
square (ACT/DVE) between two matmul passes:
    pass-1  G = U^T p          PE col-groups 0-2   [96r x 512] PSUM
    square  z = G*G (bf16)     ACT / DVE, PSUM -> SBUF
    pass-2  y += smat_o^T z    PE col-group 3, +/-1 lhsT, accumulated
                               over the 24 o's of one L-chunk in PSUM

Sharding: 8 cores = 2 image-pairs x 4 O-blocks.  Core c: images
(2g, 2g+1) with g=c//4, out-channels 24j..24j+23 with j=c%4.  DMA per
core ~0.93MB (vs 7.5MB for the replicated weight+mask baseline), which
un-gates the whole pipeline from HBM.

Per-core device pipeline: bf16 everywhere outside PSUM; pooling is 4
bf16 adds per image (img0 on DVE, img1 on GpSimd - 1/9 scales live in
U); PE warmup burst keeps the HAM clock-gate open; squares alternate
ACT/DVE; pass-2 lags pass-1 by 2 o-groups so the PE queue never stalls
on the square engines; bias rides the per-chunk PSUM->SBUF drain.
"""
import numpy as np
import ml_dtypes

import concourse.bass as bass
import concourse.bacc as bacc
import concourse.mybir as mybir
from concourse import tile
from concourse import bass_utils

C = 96
O = 96
OB = 24            # out-channels per core
L = 2048           # locations per core (2 images x 32x32)
N_CORES = 8
WARMUP_MMS = 12
F32 = mybir.dt.float32
BF16 = mybir.dt.bfloat16
BF16_NP = ml_dtypes.bfloat16

# square-engine plan per (lc, g) unit, cycled: 'A' = ACT square direct
# from PSUM; 'D'/'G' = DVE copies PSUM->SBUF bf16, then the bf16 square
# runs on DVE ('D') or GpSimd ('G').  (A TensorTensor with two PSUM
# operands is rejected by the verifier, so DVE cannot square PSUM
# directly.)
UNIT_PLAN = ["A", "A", "D", "A", "A", "D", "A", "A",
             "D", "A", "A", "D", "A", "A", "D", "A"]


def _build_kernel(nc: bass.Bass):
    xs_d = nc.dram_tensor("xs", [C, 2 * 34 * 34], BF16, kind="ExternalInput")
    u_d = nc.dram_tensor("u", [C, OB * C], BF16, kind="ExternalInput")
    smat_d = nc.dram_tensor("smat", [C, OB * OB], BF16, kind="ExternalInput")
    b_d = nc.dram_tensor("bias", [128, 1], F32, kind="ExternalInput")
    y_d = nc.dram_tensor("y", [OB, L], F32, kind="ExternalOutput")

    with tile.TileContext(nc) as tc:
        with (
            tc.tile_pool(name="const", bufs=1) as cpool,
            tc.tile_pool(name="work", bufs=1) as wpool,
            tc.tile_pool(name="z", bufs=6) as zpool,
            tc.tile_pool(name="tc16", bufs=4) as tcpool,
            tc.tile_pool(name="ysb", bufs=2) as ypool_sb,
            tc.tile_pool(name="tpsum", bufs=2, space="PSUM") as tpsum,
            tc.tile_pool(name="ypsum", bufs=2, space="PSUM") as ypsum,
        ):
            xs = cpool.tile([C, 2 * 1156], BF16)
            u16 = cpool.tile([C, OB * C], BF16)
            smat = cpool.tile([C, OB * OB], BF16)
            bias = cpool.tile([128, 1], F32)
            warm = cpool.tile([C, 512], BF16)
            zwarm = cpool.tile([C, 8], BF16)

            # DMA order: image 0 first (gates pooling/pass-1 start), then
            # the small pass-2/bias tensors, then image 1 and U in blocks.
            nc.sync.dma_start(xs[:, 0:1156], xs_d.ap()[:, 0:1156])
            nc.sync.dma_start(smat[:], smat_d.ap())
            nc.sync.dma_start(bias[:], b_d.ap())
            UB = OB * C // 4
            nc.sync.dma_start(u16[:, 0:UB], u_d.ap()[:, 0:UB])
            nc.sync.dma_start(xs[:, 1156:2312], xs_d.ap()[:, 1156:2312])
            for b in range(1, 4):
                nc.sync.dma_start(u16[:, b * UB:(b + 1) * UB],
                                  u_d.ap()[:, b * UB:(b + 1) * UB])

            nc.vector.memset(warm[:], 0.0)
            # preload the ACT Square spline tables while DMA runs
            nc.scalar.square(zwarm[:], warm[:, 0:8])

            # PE warmup: ~5us of garbage matmuls lifts the HAM clock gate
            # 1.2 -> 2.4 GHz before the real pass-1 stream begins.  The
            # garbage lands in a ypsum buffer (reused later, WAR-ordered).
            wps = ypsum.tile([128, 512], F32, tag="y_ps")
            for _ in range(WARMUP_MMS):
                nc.tensor.matmul(wps[0:C, :], warm[:, 0:C], warm[:],
                                 start=True, stop=True, skip_group_check=True)

            # --- pooling: 3x3 box sums, bf16; img0 on DVE, img1 on GpSimd
            s1 = wpool.tile([C, 2 * 34 * 33], BF16)
            s2 = wpool.tile([C, 2 * 34 * 32], BF16)
            v1 = wpool.tile([C, 2 * 33 * 32], BF16)
            pt16 = wpool.tile([C, L], BF16)
            for i, eng in ((0, nc.vector), (1, nc.gpsimd)):
                xv = xs[:, i * 1156:(i + 1) * 1156].rearrange(
                    "c (h w) -> c h w", h=34)
                s1v = s1[:, i * 1122:(i + 1) * 1122].rearrange(
                    "c (h w) -> c h w", h=34)
                s2v = s2[:, i * 1088:(i + 1) * 1088].rearrange(
                    "c (h w) -> c h w", h=34)
                v1v = v1[:, i * 1056:(i + 1) * 1056].rearrange(
                    "c (h w) -> c h w", h=33)
                ptv = pt16[:, i * 1024:(i + 1) * 1024].rearrange(
                    "c (h w) -> c h w", h=32)
                eng.tensor_add(s1v, xv[:, :, 0:33], xv[:, :, 1:34])
                eng.tensor_add(s2v, s1v[:, :, 0:32], xv[:, :, 2:34])
                eng.tensor_add(v1v, s2v[:, 0:33, :], s2v[:, 1:34, :])
                eng.tensor_add(ptv, v1v[:, 0:32, :], s2v[:, 2:34, :])

            # --- main loop: 4 L-chunks x 8 o-groups of 3 ---
            unit_idx = [0]

            for lc in range(4):
                y_ps = ypsum.tile([128, 512], F32)
                rhs = pt16[:, lc * 512:(lc + 1) * 512]
                zq = []  # pending (o, z_tile, k) for pass-2

                def flush(keep=0):
                    while len(zq) > keep:
                        o, zt, k = zq.pop(0)
                        nc.tensor.matmul(
                            y_ps[96:120, :],
                            smat[:, o * OB:(o + 1) * OB],
                            zt[:, k * 512:(k + 1) * 512],
                            start=(o == 0), stop=(o == OB - 1),
                            skip_group_check=True, tile_position=(0, 96),
                        )

                for g in range(8):
                    T = tpsum.tile([C, 3 * 512], F32)
                    for k in range(3):
                        o = 3 * g + k
                        # three M=32 matmuls on PE col-groups 0/1/2 run
                        # concurrently (one wide M=96 matmul serializes)
                        for s in range(3):
                            nc.tensor.matmul(
                                T[32 * s:32 * (s + 1), k * 512:(k + 1) * 512],
                                u16[:, o * C + 32 * s:o * C + 32 * (s + 1)],
                                rhs, start=True, stop=True,
                            )
                    flush(keep=9)
                    z = zpool.tile([C, 3 * 512], BF16, tag="z")
                    kind = UNIT_PLAN[unit_idx[0] % len(UNIT_PLAN)]
                    unit_idx[0] += 1
                    if kind == "A":
                        nc.scalar.square(z[:], T[:])
                    else:
                        tc16 = tcpool.tile([C, 3 * 512], BF16, tag="tc")
                        nc.vector.tensor_copy(tc16[:], T[:])
                        eng = nc.vector if kind == "D" else nc.gpsimd
                        eng.tensor_mul(z[:], tc16[:], tc16[:])
                    for k in range(3):
                        zq.append((3 * g + k, z, k))
                flush()
                y_sb = ypool_sb.tile([128, 512], F32)
                nc.vector.tensor_scalar_add(
                    y_sb[96:120, :], y_ps[96:120, :], bias[96:120, :])
                nc.sync.dma_start(y_d.ap()[:, lc * 512:(lc + 1) * 512],
                                  y_sb[96:120, :])

    return nc


_NC_CACHE = {}


def _get_nc():
    if "nc" not in _NC_CACHE:
        nc = bacc.Bacc("TRN2", target_bir_lowering=False, debug=False,
                       enable_asserts=False)
        _build_kernel(nc)
        nc.compile()
        _NC_CACHE["nc"] = nc
    return _NC_CACHE["nc"]


def _prep_shards(x, weight, mask, bias):
    wm = np.asarray(weight, np.float32) * np.asarray(mask, np.float32)
    S = wm + wm.transpose(0, 2, 1)
    lam, V = np.linalg.eigh(S)                       # [O, R], [O, C, R]
    U = V * (np.sqrt(np.abs(lam) / 2.0)[:, None, :] / 9.0)
    sgn = np.sign(lam).astype(np.float32)            # [O, R]

    x16 = np.asarray(x, np.float32).astype(BF16_NP)
    xp = np.pad(x16, ((0, 0), (0, 0), (1, 1), (1, 1)))   # [4, C, 34, 34]

    u_blocks, s_blocks, b_blocks = [], [], []
    bsrc = np.asarray(bias, np.float32).ravel()
    for j in range(4):
        osel = slice(OB * j, OB * (j + 1))
        # u[c, o_local*96 + r] = U[o, c, r]
        ub = np.ascontiguousarray(
            U[osel].transpose(1, 0, 2).reshape(C, OB * C)).astype(BF16_NP)
        sb = np.zeros((C, OB, OB), np.float32)
        for oo in range(OB):
            sb[:, oo, oo] = sgn[OB * j + oo]
        sb = np.ascontiguousarray(sb.reshape(C, OB * OB)).astype(BF16_NP)
        bb = np.zeros((128, 1), np.float32)
        bb[96:120, 0] = bsrc[osel]
        u_blocks.append(ub)
        s_blocks.append(sb)
        b_blocks.append(bb)

    xs_pairs = []
    for g in range(2):
        xs = np.ascontiguousarray(
            xp[2 * g:2 * g + 2].transpose(1, 0, 2, 3).reshape(C, 2 * 1156))
        xs_pairs.append(xs.astype(BF16_NP))

    in_maps = []
    for core in range(N_CORES):
        g, j = core // 4, core % 4
        in_maps.append({"xs": xs_pairs[g], "u": u_blocks[j],
                        "smat": s_blocks[j], "bias": b_blocks[j]})
    return in_maps


def run_sharded(x, weight, mask, bias, **run_kwargs):
    """Run on the 8 NeuronCores; returns (y_full, BassKernelResults)."""
    nc = _get_nc()
    in_maps = _prep_shards(x, weight, mask, bias)
    res = bass_utils.run_bass_kernel_spmd(
        nc, in_maps, core_ids=list(range(N_CORES)), **run_kwargs)
    y = np.empty((4, O, 32, 32), dtype=np.float32)
    for core in range(N_CORES):
        g, j = core // 4, core % 4
        yc = res.results[core]["y"].reshape(OB, 2, 32, 32)
        y[2 * g, OB * j:OB * (j + 1)] = yc[:, 0]
        y[2 * g + 1, OB * j:OB * (j + 1)] = yc[:, 1]
    return y, res


def kernel(x, weight, mask, bias):
    y, _ = run_sharded(x, weight, mask, bias)
    return y
